# revision 20
# baseline (speedup 1.0000x reference)
"""KNN classifier kernel for Trainium2 (8 NeuronCores, Bass/Tile).

Problem (nn_KNNClassifier): given queries x [4096, 512], train bank
x_train [65536, 512], labels y_train [65536] (100 classes), compute for
each query the top-200 neighbors by dot-product similarity, weight them
by exp(sim/0.1), accumulate per-class scores, and return the descending
argsort of class scores -> int32 [4096, 100].

Device strategy (shard train bank over N across 8 cores):
  - Host reorders x_train columns by class; each class is one contiguous
    "slot" (zero-padded to a multiple of 8). Slots are distributed
    round-robin over the 8 cores (13 slots/core); every core gets the
    full query set.
  - Inputs are bf16 (quantization error sigma ~0.035 on sims of std
    ~22.6); all weights + queries stay SBUF-resident, so the PE streams
    matmuls back-to-back at ~1 col/cycle with no mid-kernel DMA stalls.
  - Per core: sim = x @ shard^T accumulated over 4 k-slices into PSUM;
    one DVE max8 per class slot reads PSUM directly (no PSUM->SBUF
    copy) -> top-8 fp32 values per (query, class-slot).
  - Host gathers 8 * 13 * 8 candidates per query, computes the top-200
    threshold, and exactly recomputes (fp64) every slot with a candidate
    within SLACK of the threshold or whose 8th max is near it, so the
    selected top-200 set matches fp32 reference semantics.
  - Final per-class accumulation mimics the reference exactly (fp32 exp
    -> scatter-add -> stable argsort of negated scores).
"""

import os
import sys

for _p in ("/opt/trn_rl_repo",):
    if _p not in sys.path and os.path.isdir(_p):
        sys.path.insert(0, _p)

import numpy as np

import concourse.mybir as mybir
import concourse.tile as tile
from concourse import bacc
from concourse.bass_utils import run_bass_kernel_spmd

# Problem shapes (hardcoded per spec)
B, N, D = 4096, 65536, 512
NUM_CLASSES = 100
KNN_K = 200
KNN_T = 0.1
NCORES = 8

KT = D // 128  # 4 contraction tiles
QB = B // 128  # 32 query blocks of 128
GROUP_COLS = 2048  # max streamed-group width (4 PSUM banks)

SLACK = 0.40  # exact-recompute band around the top-200 threshold (bf16
# input-quantization error: sigma ~0.053, max over 268M sims ~0.34)
NEG = -1.0e30

_CACHE = {}
LAST_INFO = {}


def _build_program(groups):
    """Per-core Bass program.

    groups[i] is the list of slot widths in psum-group i. Slots are
    class-pure column ranges; each gets one DVE max8 directly on PSUM.
    Matmuls within a group use n-tiles of 512 columns (last one ragged),
    each inside its own PSUM bank, so sim data is contiguous per group.
    All weights and queries are bf16 and SBUF-resident.
    """
    nc = bacc.Bacc(
        "TRN2", target_bir_lowering=False, debug=False, num_devices=NCORES
    )
    f32 = mybir.dt.float32
    bf16 = mybir.dt.bfloat16

    ncols = sum(sum(g) for g in groups)
    nslots = sum(len(g) for g in groups)
    cands = nslots * 8

    xT_d = nc.dram_tensor("xT", (D, B), bf16, kind="ExternalInput").ap()
    wT_d = nc.dram_tensor("wT", (D, ncols), bf16, kind="ExternalInput").ap()
    vals_d = nc.dram_tensor("vals", (B, cands), f32, kind="ExternalOutput").ap()

    from contextlib import ExitStack

    with tile.TileContext(nc) as tc:
        with ExitStack() as ctx:
            xpool = ctx.enter_context(tc.tile_pool(name="xp", bufs=1))
            wpool = ctx.enter_context(tc.tile_pool(name="wp", bufs=1))
            ppool = ctx.enter_context(tc.tile_pool(name="pp", bufs=2, space="PSUM"))
            opool = ctx.enter_context(tc.tile_pool(name="op", bufs=4))

            xsb = xpool.tile([128, KT * B], bf16, tag="x")
            wsb = wpool.tile([128, KT * ncols], bf16, tag="w")



            # DMA schedule: the first matmul (g0, b0, k0, nt0) needs only
            # 512 weight cols + 512 query cols, so chunk the initial
            # transfers finely and order them by first use. Everything
            # else prefetches behind on the scalar engine's DMA queue so
            # it cannot delay the critical early transfers (sync queue)
            # or the vals outputs (gpsimd queue).
            g0 = sum(groups[0])
            XC = 2048  # steady x chunk in queries (4KB DMA lines)
            # First b-block consumes k-outer: for each k it needs all of
            # w[g0, k] plus x[k, b0] (128 queries only). Emit exactly
            # that on the sync (hardware) queue so the cold-start PE
            # never waits; the remaining weight groups prefetch on the
            # scalar queue and the vals outputs use the gpsimd queue, so
            # neither can delay the critical early transfers.
            for k in range(KT):
                nc.sync.dma_start(
                    xsb[:, k * B : k * B + 128],
                    xT_d[k * 128 : (k + 1) * 128, 0:128],
                )
                # 1024-col w chunks: earlier deps for the k-stage matmuls
                # while keeping 2KB DMA lines (512-col chunks are slower).
                for t0 in range(0, g0, 1024):
                    t1 = min(t0 + 1024, g0)
                    nc.sync.dma_start(
                        wsb[:, k * ncols + t0 : k * ncols + t1],
                        wT_d[k * 128 : (k + 1) * 128, t0:t1],
                    )
            for c in range(128, B, XC):
                ce = min(c + XC, B)
                for k in range(KT):
                    nc.sync.dma_start(
                        xsb[:, k * B + c : k * B + ce],
                        xT_d[k * 128 : (k + 1) * 128, c:ce],
                    )
            col0 = g0
            for gi in range(1, len(groups)):
                gcols = sum(groups[gi])
                for k in range(KT):
                    nc.scalar.dma_start(
                        wsb[:, k * ncols + col0 : k * ncols + col0 + gcols],
                        wT_d[k * 128 : (k + 1) * 128, col0 : col0 + gcols],
                    )
                col0 += gcols

            col0 = 0  # start column of current group
            slot0 = 0  # first slot index of current group
            for gi, gslots in enumerate(groups):
                gcols = sum(gslots)
                tiles = [512] * (gcols // 512)
                if gcols % 512:
                    tiles.append(gcols % 512)
                gnt = len(tiles)
                for b in range(QB):
                    # n-tiles are 512 wide (bank-aligned, last ragged), so
                    # psum/sim data is contiguous over [0, gcols).
                    ps = ppool.tile([128, gnt * 512], f32, tag="ps")
                    for k in range(KT):
                        toff = 0
                        for nt, ntw in enumerate(tiles):
                            nc.tensor.matmul(
                                ps[:, toff : toff + ntw],
                                xsb[:, k * B + b * 128 : k * B + (b + 1) * 128],
                                wsb[
                                    :,
                                    k * ncols + col0 + toff : k * ncols
                                    + col0
                                    + toff
                                    + ntw,
                                ],
                                start=(k == 0),
                                stop=(k == KT - 1),
                            )
                            toff += ntw
                    vt = opool.tile([128, len(gslots) * 8], f32, tag="v")
                    soff = 0
                    for si, sw in enumerate(gslots):
                        nc.vector.max(
                            vt[:, si * 8 : (si + 1) * 8],
                            ps[:, soff : soff + sw],
                        )
                        soff += sw
                    nc.gpsimd.dma_start(
                        vals_d[
                            b * 128 : (b + 1) * 128,
                            slot0 * 8 : (slot0 + len(gslots)) * 8,
                        ],
                        vt[:],
                    )
                col0 += gcols
                slot0 += len(gslots)

    nc.compile()
    return nc


def _get_program(groups):
    key = tuple(tuple(g) for g in groups)
    if key not in _CACHE:
        _CACHE[key] = _build_program(groups)
    return _CACHE[key]


def _plan_layout(y_train):
    """Class-pure slot layout, identical structure on all cores.

    Each class is one piece; pieces are sorted by width and packed
    8-at-a-time into "columns": column g holds one piece per core,
    zero-padded to the widest piece of its group (rounded to 8). Each
    (core, column) is a single-class slot covered by one DVE max8.
    Columns are then packed into streaming groups of <= GROUP_COLS.

    Returns (colmap, slot_class, slot_start, slot_width, groups):
      colmap: int64 [8 * cols_per_core] -> original x_train row, -1 pad
      slot_class/start/width: int64 [8 * S], device slot order, core-major
      groups: per-core streaming groups as lists of slot widths
    """
    cnt = np.bincount(y_train, minlength=NUM_CLASSES)
    by_class = np.argsort(y_train, kind="stable")  # rows grouped by class
    starts = np.zeros(NUM_CLASSES + 1, dtype=np.int64)
    np.cumsum(cnt, out=starts[1:])

    pieces = []  # (width, class, offset in by_class)
    for c in range(NUM_CLASSES):
        n = int(cnt[c])
        splits = 2
        while (n + splits - 1) // splits > GROUP_COLS:
            splits *= 2
        off = int(starts[c])
        base, rem = divmod(n, splits)
        for s in range(splits):
            w = base + (1 if s < rem else 0)
            pieces.append((w, c, off))
            off += w
    while len(pieces) % NCORES:
        pieces.append((0, -1, 0))
    pieces.sort(key=lambda p: -p[0])

    S = len(pieces) // NCORES  # slots (columns) per core
    colw = [((max(pieces[g * NCORES][0], 1) + 7) // 8) * 8 for g in range(S)]

    packed = _pack_groups(colw)  # groups of column ids
    dev_order = [g for grp in packed for g in grp]
    groups = [[colw[g] for g in grp] for grp in packed]
    cols_per_core = sum(colw)

    colmap = np.full(NCORES * cols_per_core, -1, dtype=np.int64)
    slot_class = np.full(NCORES * S, -1, dtype=np.int64)
    slot_start = np.zeros(NCORES * S, dtype=np.int64)
    slot_width = np.zeros(NCORES * S, dtype=np.int64)
    off_in_core = 0
    for j, g in enumerate(dev_order):  # j = device slot position
        w = colw[g]
        for i in range(NCORES):
            pw, c, poff = pieces[g * NCORES + i]
            gs = i * S + j  # global slot id (core-major, device order)
            col = i * cols_per_core + off_in_core
            slot_class[gs] = c
            slot_start[gs] = col
            slot_width[gs] = w
            if pw:
                colmap[col : col + pw] = by_class[poff : poff + pw]
        off_in_core += w

    return colmap, slot_class, slot_start, slot_width, groups


def _pack_groups(widths):
    """Partition column ids into groups with sum <= GROUP_COLS, preferring
    groups whose (sum mod 512) is 0 or >= 452."""
    remaining = sorted(range(len(widths)), key=lambda i: -widths[i])
    groups = []
    while remaining:
        cur = [remaining.pop(0)]
        tot = widths[cur[0]]
        while True:
            cands = [i for i, g in enumerate(remaining) if tot + widths[g] <= GROUP_COLS]
            if not cands:
                break

            def score(i):
                t = tot + widths[remaining[i]]
                r = t % 512
                return (0 if (r == 0 or r >= 452) else 1, -t)

            i = min(cands, key=score)
            tot += widths[remaining[i]]
            cur.append(remaining.pop(i))
        groups.append(cur)
    return groups


def _host_merge(x, x_train, y_train, vals, colmap, slot_class, slot_start, slot_width):
    """Exact top-200 -> class scores -> ranking from per-core candidates."""
    x64 = x.astype(np.float64)
    xt64 = x_train.astype(np.float64)
    TS = slot_class.shape[0]  # global slot count
    M = TS * 8

    V = np.concatenate(list(vals), axis=1).astype(np.float64)  # [B, M]
    V[V == 0.0] = NEG  # zero-pad artifacts (real sims are never exactly 0)

    kth = M - KNN_K
    t0 = np.partition(V, kth, axis=1)[:, kth]  # [B] approx threshold

    # Slots needing exact recomputation: any candidate within SLACK of
    # the threshold, or slot 8th-max (possible hidden elements) near it.
    band = (V >= (t0[:, None] - SLACK - 0.01)) & (V <= (t0[:, None] + SLACK))
    v8 = V.reshape(B, TS, 8)[:, :, 7]
    flag = v8 >= (t0[:, None] - SLACK)  # slot may hide >8 relevant entries
    slot_band = band.reshape(B, TS, 8).any(axis=2) | flag  # [B, TS]

    bq, bg = np.nonzero(slot_band)
    LAST_INFO["recomputed_chunks"] = int(bq.size)
    full_fallback = set()
    if bq.size:
        # Exact sims per (query, slot) pair, grouped by slot so each
        # slot's column matrix is gathered and transposed only once.
        Vr = V.reshape(B, TS, 8)
        order = np.argsort(bg, kind="stable")
        bq_s, bg_s = bq[order], bg[order]
        uniq, starts = np.unique(bg_s, return_index=True)
        bounds = list(starts) + [bg_s.size]
        for i in range(len(uniq)):
            s, e = bounds[i], bounds[i + 1]
            g = int(uniq[i])
            qs = bq_s[s:e]
            c0 = int(slot_start[g])
            w = int(slot_width[g])
            rows = colmap[c0 : c0 + w]
            pad = rows < 0
            Wg = xt64[np.where(pad, 0, rows)].T  # [D, w]
            exact = x64[qs] @ Wg  # [nq, w]
            exact[:, pad] = NEG
            thr = t0[qs] - SLACK - 0.005
            nkeep = (exact >= thr[:, None]).sum(axis=1)
            top8 = -np.sort(-exact, axis=1)[:, :8]
            Vr[qs, g] = top8
            for q in qs[nkeep > 8]:
                full_fallback.add(int(q))

    t1 = np.partition(V, kth, axis=1)[:, kth]
    sel = np.argpartition(-V, KNN_K - 1, axis=1)[:, :KNN_K]
    rowix = np.arange(B)[:, None]
    sel_v = V[rowix, sel]

    # Boundary ties -> per-query fallback (argpartition splits arbitrarily)
    vmin = sel_v.min(axis=1)
    tie = (V == vmin[:, None]).sum(axis=1) != (sel_v == vmin[:, None]).sum(axis=1)
    for q in np.nonzero(tie)[0]:
        full_fallback.add(int(q))

    # Pathological guard: if the top-200 threshold ever sits near/below 0,
    # zero-pad dropping could hide real candidates -> recompute those rows.
    for q in np.nonzero(t1 < 1.0)[0]:
        full_fallback.add(int(q))
    LAST_INFO["fallback_rows"] = len(full_fallback)

    cand_class = np.repeat(slot_class, 8)  # [M] class per candidate slot
    labels = cand_class[sel]  # [B, K]

    scores = np.zeros((B, NUM_CLASSES), dtype=np.float32)
    with np.errstate(over="ignore"):
        w = np.exp(sel_v.astype(np.float32) / np.float32(KNN_T))
    ok = np.ones(B, dtype=bool)
    for q in full_fallback:
        ok[q] = False
    qs = np.nonzero(ok)[0]
    np.add.at(
        scores,
        (np.repeat(qs, KNN_K), labels[qs].ravel()),
        w[qs].ravel(),
    )

    for q in full_fallback:
        sims = xt64 @ x64[q]
        order = np.lexsort((np.arange(N), -sims))[:KNN_K]
        lab = y_train[order]
        with np.errstate(over="ignore"):
            wq = np.exp(sims[order].astype(np.float32) / np.float32(KNN_T))
        np.add.at(scores[q], lab, wq)

    return np.argsort(-scores, axis=1, kind="stable").astype(np.int32)


def kernel(x, x_train, y_train):
    import ml_dtypes

    bf16 = ml_dtypes.bfloat16

    x = np.asarray(x, dtype=np.float32)
    x_train = np.asarray(x_train, dtype=np.float32)
    y_train = np.asarray(y_train).astype(np.int64)

    colmap, slot_class, slot_start, slot_width, groups = _plan_layout(y_train)
    nc = _get_program(groups)

    ncols_tot = colmap.shape[0]
    ncols = ncols_tot // NCORES
    xtrP = np.zeros((D, ncols_tot), dtype=np.float32)  # padded, transposed
    real = colmap >= 0
    xtrP[:, real] = x_train.T[:, colmap[real]]
    xtrP16 = xtrP.astype(bf16)

    xT = np.ascontiguousarray(x.T).astype(bf16)
    in_maps = [
        {
            "xT": xT,
            "wT": np.ascontiguousarray(xtrP16[:, c * ncols : (c + 1) * ncols]),
        }
        for c in range(NCORES)
    ]

    res = run_bass_kernel_spmd(nc, in_maps, core_ids=list(range(NCORES)))
    LAST_INFO["exec_time_ns"] = res.exec_time_ns
    LAST_INFO["results"] = res

    vals = np.stack([res.results[c]["vals"] for c in range(NCORES)])
    LAST_INFO["vals"] = vals
    LAST_INFO["layout"] = (colmap, slot_class, slot_start, slot_width)
    return _host_merge(
        x, x_train, y_train, vals, colmap, slot_class, slot_start, slot_width
    )


# revision 22
# speedup vs baseline: 1.0279x; 1.0279x over previous
"""KNN classifier kernel for Trainium2 (8 NeuronCores, Bass/Tile).

Problem (nn_KNNClassifier): given queries x [4096, 512], train bank
x_train [65536, 512], labels y_train [65536] (100 classes), compute for
each query the top-200 neighbors by dot-product similarity, weight them
by exp(sim/0.1), accumulate per-class scores, and return the descending
argsort of class scores -> int32 [4096, 100].

Device strategy (shard train bank over N across 8 cores):
  - Host reorders x_train columns by class; each class is split into two
    contiguous "slots" (zero-padded to a multiple of 8, ~330 cols).
    Slots are distributed round-robin over the 8 cores (25 slots/core);
    every core gets the full query set.
  - Inputs are bf16 (quantization error sigma ~0.053 on sims of std
    ~22.6); all weights + queries stay SBUF-resident, so the PE streams
    matmuls back-to-back at ~1 col/cycle with no mid-kernel DMA stalls.
  - Per core: sim = x @ shard^T accumulated over 4 k-slices into PSUM;
    one DVE max8 per slot reads PSUM directly (no PSUM->SBUF copy)
    -> top-8 fp32 values per (query, slot).
  - Host gathers 8 * 25 * 8 candidates per query, computes the top-200
    threshold, and exactly recomputes (fp64) every slot with a candidate
    within SLACK of the threshold or whose 8th max is near it, so the
    selected top-200 set matches fp32 reference semantics.
  - Final per-class accumulation mimics the reference exactly (fp32 exp
    -> scatter-add -> stable argsort of negated scores).
"""

import os
import sys

for _p in ("/opt/trn_rl_repo",):
    if _p not in sys.path and os.path.isdir(_p):
        sys.path.insert(0, _p)

import numpy as np

import concourse.mybir as mybir
import concourse.tile as tile
from concourse import bacc
from concourse.bass_utils import run_bass_kernel_spmd

# Problem shapes (hardcoded per spec)
B, N, D = 4096, 65536, 512
NUM_CLASSES = 100
KNN_K = 200
KNN_T = 0.1
NCORES = 8

KT = D // 128  # 4 contraction tiles
QB = B // 128  # 32 query blocks of 128
GROUP_COLS = 2048  # max streamed-group width (4 PSUM banks)

SLACK = 0.40  # exact-recompute band around the top-200 threshold (bf16
# input-quantization error: sigma ~0.053, max over 268M sims ~0.34)
NEG = -1.0e30

_CACHE = {}
LAST_INFO = {}


def _build_program(groups):
    """Per-core Bass program.

    groups[i] is the list of slot widths in psum-group i. Slots are
    class-pure column ranges; each gets one DVE max8 directly on PSUM.
    Matmuls within a group use n-tiles of 512 columns (last one ragged),
    each inside its own PSUM bank, so sim data is contiguous per group.
    All weights and queries are bf16 and SBUF-resident.
    """
    nc = bacc.Bacc(
        "TRN2", target_bir_lowering=False, debug=False, num_devices=NCORES
    )
    f32 = mybir.dt.float32
    bf16 = mybir.dt.bfloat16

    ncols = sum(sum(g) for g in groups)
    nslots = sum(len(g) for g in groups)
    cands = nslots * 8

    xT_d = nc.dram_tensor("xT", (D, B), bf16, kind="ExternalInput").ap()
    wT_d = nc.dram_tensor("wT", (D, ncols), bf16, kind="ExternalInput").ap()
    vals_d = nc.dram_tensor("vals", (B, cands), f32, kind="ExternalOutput").ap()

    from contextlib import ExitStack

    with tile.TileContext(nc) as tc:
        with ExitStack() as ctx:
            xpool = ctx.enter_context(tc.tile_pool(name="xp", bufs=1))
            wpool = ctx.enter_context(tc.tile_pool(name="wp", bufs=1))
            ppool = ctx.enter_context(tc.tile_pool(name="pp", bufs=2, space="PSUM"))
            opool = ctx.enter_context(tc.tile_pool(name="op", bufs=4))

            xsb = xpool.tile([128, KT * B], bf16, tag="x")
            wsb = wpool.tile([128, KT * ncols], bf16, tag="w")



            # DMA schedule: the first matmul (g0, b0, k0, nt0) needs only
            # 512 weight cols + 512 query cols, so chunk the initial
            # transfers finely and order them by first use. Everything
            # else prefetches behind on the scalar engine's DMA queue so
            # it cannot delay the critical early transfers (sync queue)
            # or the vals outputs (gpsimd queue).
            g0 = sum(groups[0])
            XC = 512  # steady x chunk in queries
            # First b-block consumes k-outer: for each k it needs all of
            # w[g0, k] plus x[k, b0] (128 queries only). Emit exactly
            # that on the sync (hardware) queue so the cold-start PE
            # never waits; the remaining weight groups prefetch on the
            # scalar queue and the vals outputs use the gpsimd queue, so
            # neither can delay the critical early transfers.
            for k in range(KT):
                nc.sync.dma_start(
                    xsb[:, k * B : k * B + 128],
                    xT_d[k * 128 : (k + 1) * 128, 0:128],
                )
                nc.sync.dma_start(
                    wsb[:, k * ncols : k * ncols + g0],
                    wT_d[k * 128 : (k + 1) * 128, 0:g0],
                )
            for c in range(128, B, XC):
                ce = min(c + XC, B)
                for k in range(KT):
                    nc.sync.dma_start(
                        xsb[:, k * B + c : k * B + ce],
                        xT_d[k * 128 : (k + 1) * 128, c:ce],
                    )
            col0 = g0
            for gi in range(1, len(groups)):
                gcols = sum(groups[gi])
                for k in range(KT):
                    nc.scalar.dma_start(
                        wsb[:, k * ncols + col0 : k * ncols + col0 + gcols],
                        wT_d[k * 128 : (k + 1) * 128, col0 : col0 + gcols],
                    )
                col0 += gcols

            col0 = 0  # start column of current group
            slot0 = 0  # first slot index of current group
            for gi, gslots in enumerate(groups):
                gcols = sum(gslots)
                tiles = [512] * (gcols // 512)
                if gcols % 512:
                    tiles.append(gcols % 512)
                gnt = len(tiles)
                for b in range(QB):
                    # n-tiles are 512 wide (bank-aligned, last ragged), so
                    # psum/sim data is contiguous over [0, gcols).
                    ps = ppool.tile([128, gnt * 512], f32, tag="ps")
                    for k in range(KT):
                        toff = 0
                        for nt, ntw in enumerate(tiles):
                            nc.tensor.matmul(
                                ps[:, toff : toff + ntw],
                                xsb[:, k * B + b * 128 : k * B + (b + 1) * 128],
                                wsb[
                                    :,
                                    k * ncols + col0 + toff : k * ncols
                                    + col0
                                    + toff
                                    + ntw,
                                ],
                                start=(k == 0),
                                stop=(k == KT - 1),
                            )
                            toff += ntw
                    vt = opool.tile([128, len(gslots) * 8], f32, tag="v")
                    soff = 0
                    for si, sw in enumerate(gslots):
                        nc.vector.max(
                            vt[:, si * 8 : (si + 1) * 8],
                            ps[:, soff : soff + sw],
                        )
                        soff += sw
                    nc.gpsimd.dma_start(
                        vals_d[
                            b * 128 : (b + 1) * 128,
                            slot0 * 8 : (slot0 + len(gslots)) * 8,
                        ],
                        vt[:],
                    )
                col0 += gcols
                slot0 += len(gslots)

    nc.compile()
    return nc


def _get_program(groups):
    key = tuple(tuple(g) for g in groups)
    if key not in _CACHE:
        _CACHE[key] = _build_program(groups)
    return _CACHE[key]


def _plan_layout(y_train):
    """Class-pure slot layout, identical structure on all cores.

    Each class is one piece; pieces are sorted by width and packed
    8-at-a-time into "columns": column g holds one piece per core,
    zero-padded to the widest piece of its group (rounded to 8). Each
    (core, column) is a single-class slot covered by one DVE max8.
    Columns are then packed into streaming groups of <= GROUP_COLS.

    Returns (colmap, slot_class, slot_start, slot_width, groups):
      colmap: int64 [8 * cols_per_core] -> original x_train row, -1 pad
      slot_class/start/width: int64 [8 * S], device slot order, core-major
      groups: per-core streaming groups as lists of slot widths
    """
    cnt = np.bincount(y_train, minlength=NUM_CLASSES)
    by_class = np.argsort(y_train, kind="stable")  # rows grouped by class
    starts = np.zeros(NUM_CLASSES + 1, dtype=np.int64)
    np.cumsum(cnt, out=starts[1:])

    pieces = []  # (width, class, offset in by_class)
    for c in range(NUM_CLASSES):
        n = int(cnt[c])
        splits = 2
        while (n + splits - 1) // splits > GROUP_COLS:
            splits *= 2
        off = int(starts[c])
        base, rem = divmod(n, splits)
        for s in range(splits):
            w = base + (1 if s < rem else 0)
            pieces.append((w, c, off))
            off += w
    while len(pieces) % NCORES:
        pieces.append((0, -1, 0))
    pieces.sort(key=lambda p: -p[0])

    S = len(pieces) // NCORES  # slots (columns) per core
    colw = [((max(pieces[g * NCORES][0], 1) + 7) // 8) * 8 for g in range(S)]

    packed = _pack_groups(colw)  # groups of column ids
    dev_order = [g for grp in packed for g in grp]
    groups = [[colw[g] for g in grp] for grp in packed]
    cols_per_core = sum(colw)

    colmap = np.full(NCORES * cols_per_core, -1, dtype=np.int64)
    slot_class = np.full(NCORES * S, -1, dtype=np.int64)
    slot_start = np.zeros(NCORES * S, dtype=np.int64)
    slot_width = np.zeros(NCORES * S, dtype=np.int64)
    off_in_core = 0
    for j, g in enumerate(dev_order):  # j = device slot position
        w = colw[g]
        for i in range(NCORES):
            pw, c, poff = pieces[g * NCORES + i]
            gs = i * S + j  # global slot id (core-major, device order)
            col = i * cols_per_core + off_in_core
            slot_class[gs] = c
            slot_start[gs] = col
            slot_width[gs] = w
            if pw:
                colmap[col : col + pw] = by_class[poff : poff + pw]
        off_in_core += w

    return colmap, slot_class, slot_start, slot_width, groups


def _pack_groups(widths):
    """Partition column ids into groups with sum <= GROUP_COLS, preferring
    groups whose (sum mod 512) is 0 or >= 452."""
    remaining = sorted(range(len(widths)), key=lambda i: -widths[i])
    groups = []
    while remaining:
        cur = [remaining.pop(0)]
        tot = widths[cur[0]]
        while True:
            cands = [i for i, g in enumerate(remaining) if tot + widths[g] <= GROUP_COLS]
            if not cands:
                break

            def score(i):
                t = tot + widths[remaining[i]]
                r = t % 512
                return (0 if (r == 0 or r >= 452) else 1, -t)

            i = min(cands, key=score)
            tot += widths[remaining[i]]
            cur.append(remaining.pop(i))
        groups.append(cur)
    return groups


def _host_merge(x, x_train, y_train, vals, colmap, slot_class, slot_start, slot_width):
    """Exact top-200 -> class scores -> ranking from per-core candidates."""
    x64 = x.astype(np.float64)
    xt64 = x_train.astype(np.float64)
    TS = slot_class.shape[0]  # global slot count
    M = TS * 8

    V = np.concatenate(list(vals), axis=1).astype(np.float64)  # [B, M]
    V[V == 0.0] = NEG  # zero-pad artifacts (real sims are never exactly 0)

    kth = M - KNN_K
    t0 = np.partition(V, kth, axis=1)[:, kth]  # [B] approx threshold

    # Slots needing exact recomputation: any candidate within SLACK of
    # the threshold, or slot 8th-max (possible hidden elements) near it.
    band = (V >= (t0[:, None] - SLACK - 0.01)) & (V <= (t0[:, None] + SLACK))
    v8 = V.reshape(B, TS, 8)[:, :, 7]
    flag = v8 >= (t0[:, None] - SLACK)  # slot may hide >8 relevant entries
    slot_band = band.reshape(B, TS, 8).any(axis=2) | flag  # [B, TS]

    bq, bg = np.nonzero(slot_band)
    LAST_INFO["recomputed_chunks"] = int(bq.size)
    full_fallback = set()
    if bq.size:
        # Exact sims per (query, slot) pair, grouped by slot so each
        # slot's column matrix is gathered and transposed only once.
        Vr = V.reshape(B, TS, 8)
        order = np.argsort(bg, kind="stable")
        bq_s, bg_s = bq[order], bg[order]
        uniq, starts = np.unique(bg_s, return_index=True)
        bounds = list(starts) + [bg_s.size]
        for i in range(len(uniq)):
            s, e = bounds[i], bounds[i + 1]
            g = int(uniq[i])
            qs = bq_s[s:e]
            c0 = int(slot_start[g])
            w = int(slot_width[g])
            rows = colmap[c0 : c0 + w]
            pad = rows < 0
            Wg = xt64[np.where(pad, 0, rows)].T  # [D, w]
            exact = x64[qs] @ Wg  # [nq, w]
            exact[:, pad] = NEG
            thr = t0[qs] - SLACK - 0.005
            nkeep = (exact >= thr[:, None]).sum(axis=1)
            top8 = -np.sort(-exact, axis=1)[:, :8]
            Vr[qs, g] = top8
            for q in qs[nkeep > 8]:
                full_fallback.add(int(q))

    t1 = np.partition(V, kth, axis=1)[:, kth]
    sel = np.argpartition(-V, KNN_K - 1, axis=1)[:, :KNN_K]
    rowix = np.arange(B)[:, None]
    sel_v = V[rowix, sel]

    # Boundary ties -> per-query fallback (argpartition splits arbitrarily)
    vmin = sel_v.min(axis=1)
    tie = (V == vmin[:, None]).sum(axis=1) != (sel_v == vmin[:, None]).sum(axis=1)
    for q in np.nonzero(tie)[0]:
        full_fallback.add(int(q))

    # Pathological guard: if the top-200 threshold ever sits near/below 0,
    # zero-pad dropping could hide real candidates -> recompute those rows.
    for q in np.nonzero(t1 < 1.0)[0]:
        full_fallback.add(int(q))
    LAST_INFO["fallback_rows"] = len(full_fallback)

    cand_class = np.repeat(slot_class, 8)  # [M] class per candidate slot
    labels = cand_class[sel]  # [B, K]

    scores = np.zeros((B, NUM_CLASSES), dtype=np.float32)
    with np.errstate(over="ignore"):
        w = np.exp(sel_v.astype(np.float32) / np.float32(KNN_T))
    ok = np.ones(B, dtype=bool)
    for q in full_fallback:
        ok[q] = False
    qs = np.nonzero(ok)[0]
    np.add.at(
        scores,
        (np.repeat(qs, KNN_K), labels[qs].ravel()),
        w[qs].ravel(),
    )

    for q in full_fallback:
        sims = xt64 @ x64[q]
        order = np.lexsort((np.arange(N), -sims))[:KNN_K]
        lab = y_train[order]
        with np.errstate(over="ignore"):
            wq = np.exp(sims[order].astype(np.float32) / np.float32(KNN_T))
        np.add.at(scores[q], lab, wq)

    return np.argsort(-scores, axis=1, kind="stable").astype(np.int32)


def kernel(x, x_train, y_train):
    import ml_dtypes

    bf16 = ml_dtypes.bfloat16

    x = np.asarray(x, dtype=np.float32)
    x_train = np.asarray(x_train, dtype=np.float32)
    y_train = np.asarray(y_train).astype(np.int64)

    colmap, slot_class, slot_start, slot_width, groups = _plan_layout(y_train)
    nc = _get_program(groups)

    ncols_tot = colmap.shape[0]
    ncols = ncols_tot // NCORES
    xtrP = np.zeros((D, ncols_tot), dtype=np.float32)  # padded, transposed
    real = colmap >= 0
    xtrP[:, real] = x_train.T[:, colmap[real]]
    xtrP16 = xtrP.astype(bf16)

    xT = np.ascontiguousarray(x.T).astype(bf16)
    in_maps = [
        {
            "xT": xT,
            "wT": np.ascontiguousarray(xtrP16[:, c * ncols : (c + 1) * ncols]),
        }
        for c in range(NCORES)
    ]

    res = run_bass_kernel_spmd(nc, in_maps, core_ids=list(range(NCORES)))
    LAST_INFO["exec_time_ns"] = res.exec_time_ns
    LAST_INFO["results"] = res

    vals = np.stack([res.results[c]["vals"] for c in range(NCORES)])
    LAST_INFO["vals"] = vals
    LAST_INFO["layout"] = (colmap, slot_class, slot_start, slot_width)
    return _host_merge(
        x, x_train, y_train, vals, colmap, slot_class, slot_start, slot_width
    )
